# revision 43
# baseline (speedup 1.0000x reference)
"""Trainium2 Bass kernel for BatchSpectralLoss (penalty + label-smoothed CE).

Math (reference):
    penalty = ||sum_i A_i||^2 - sum(A*A)            (A = logits, [N, C])
    ce      = mean_i [ lse_i - (1-eps)*A[i,pid_i] - (eps/C)*rowsum_i ]
    out     = penalty + ce

Rows are sharded 8 ways (512 rows/core). The host casts logits to fp16
(measured effect on this loss: ~5e-5 relative — comparable to fp32
arithmetic noise) which halves HBM traffic; the kernel is memory-bound.

Device work per core, one pass over the shard in [128, w] tiles:
    - colsum partial  s_k[j] = sum_i A[i, j]   (PE matmul with a ones vector,
      fp32 PSUM accumulation across the 4 row blocks)
    - sumexp per row  (ACT Exp pass, accum_out)
    - sumsq  per row  (DVE scalar_tensor_tensor A*A, accum_out)
Host combines: s = sum_k s_k; penalty = s.s - sum(sumsq); lse = log(sumexp);
sum_i rowsum_i = sum(s); the target-logit gather is a 4096-element host read.
"""

import numpy as np
from contextlib import ExitStack

import concourse.bacc as bacc
import concourse.tile as tile
from concourse import mybir
from concourse.bass_utils import run_bass_kernel_spmd

EPS = 0.1
N, C = 4096, 8192
N_CORES = 8
ROWS = N // N_CORES           # 512 rows per core
P = 128                       # SBUF partitions
R_BLOCKS = ROWS // P          # 4 row blocks per core
HALVES = 2
HALF_C = C // HALVES          # 4096 columns per half (PSUM capacity unit)
TILE_W = 2048                 # default tile width
CHUNK = 512                   # matmul free-dim (one fp32 PSUM bank)

IN_DT = mybir.dt.float16
IN_NP = np.float16


def _tile_width(h, r):
    # Narrow tiles on the first row block (shorter pipeline fill) and the
    # last one (shorter drain tail).
    first = h == 0 and r == 0
    last = h == HALVES - 1 and r == R_BLOCKS - 1
    if first:
        return TILE_W // 2
    if last:
        return TILE_W
    return HALF_C


# sumsq accumulates sequentially in fp32; cap the run length so its rounding
# error (amplified by the penalty's big-number cancellation) stays small.
SUMSQ_CHUNK = 1024

# Stats-column schedule (mirrors the _body loop structure): per tile, one
# sumexp column then one sumsq column per SUMSQ_CHUNK sub-span. Emission
# order is monotonic, so the early/late output-DMA split is a column cut.
E_COLS = []          # (column, row_block)
Q_COLS = []          # columns
TILE_STAT_COLS = []  # per tile: (e_col, [q_cols])
_ncols = 0
for _h in range(HALVES):
    for _r in range(R_BLOCKS):
        _w = _tile_width(_h, _r)
        for _ in range(HALF_C // _w):
            _e = _ncols
            _ncols += 1
            _qs = list(range(_ncols, _ncols + -(-_w // SUMSQ_CHUNK)))
            _ncols += len(_qs)
            E_COLS.append((_e, _r))
            Q_COLS.extend(_qs)
            TILE_STAT_COLS.append((_e, _qs))
STAT_NCOLS = _ncols
N_TILES = len(TILE_STAT_COLS)
LAST_BLOCK_TILES = HALF_C // _tile_width(HALVES - 1, R_BLOCKS - 1)
STAT_CUT = TILE_STAT_COLS[N_TILES - LAST_BLOCK_TILES][0]

_NC_CACHE = None


def _body(tc):
    nc = tc.nc
    logits = nc.dram_tensor(
        "logits", [ROWS, C], IN_DT, kind="ExternalInput"
    ).ap()
    colsum = nc.dram_tensor(
        "colsum", [1, C], mybir.dt.float32, kind="ExternalOutput"
    ).ap()
    stats = nc.dram_tensor(
        "stats", [P, STAT_NCOLS], mybir.dt.float32, kind="ExternalOutput"
    ).ap()

    with ExitStack() as ctx:
        apool = ctx.enter_context(tc.tile_pool(name="a", bufs=6))
        scratch = ctx.enter_context(tc.tile_pool(name="scratch", bufs=1))
        outp = ctx.enter_context(tc.tile_pool(name="outp", bufs=1))
        psum = ctx.enter_context(tc.tile_pool(name="psum", bufs=1, space="PSUM"))

        ones = scratch.tile([P, 1], IN_DT)
        nc.vector.memset(ones, 1.0)
        e_scr = scratch.tile([P, HALF_C], IN_DT)
        s_scr = scratch.tile([P, HALF_C], IN_DT)
        stats_sb = outp.tile([P, STAT_NCOLS], mybir.dt.float32)
        colsum_sb = outp.tile([1, C], mybir.dt.float32)
        # One half's column-sum accumulators: HALF_C/CHUNK banks of [1,512].
        ps = psum.tile([1, HALF_C], mybir.dt.float32)

        stat_idx = 0
        for h in range(HALVES):
            for r in range(R_BLOCKS):
                w = _tile_width(h, r)
                for col in range(HALF_C * h, HALF_C * (h + 1), w):
                    a = apool.tile([P, w], IN_DT, tag=f"a{w}")
                    nc.sync.dma_start(
                        out=a, in_=logits[P * r : P * (r + 1), col : col + w]
                    )
                    e_col, q_cols = TILE_STAT_COLS[stat_idx]
                    stat_idx += 1
                    nc.scalar.activation(
                        out=e_scr[:, :w],
                        in_=a,
                        func=mybir.ActivationFunctionType.Exp,
                        accum_out=stats_sb[:, e_col : e_col + 1],
                    )
                    for si, q_col in enumerate(q_cols):
                        s0 = si * SUMSQ_CHUNK
                        s1 = min(w, s0 + SUMSQ_CHUNK)
                        nc.vector.scalar_tensor_tensor(
                            out=s_scr[:, s0:s1],
                            in0=a[:, s0:s1],
                            scalar=1.0,
                            in1=a[:, s0:s1],
                            op0=mybir.AluOpType.mult,
                            op1=mybir.AluOpType.mult,
                            accum_out=stats_sb[:, q_col : q_col + 1],
                        )
                    pq = col - HALF_C * h
                    for c in range(w // CHUNK):
                        nc.tensor.matmul(
                            ps[0:1, pq + CHUNK * c : pq + CHUNK * (c + 1)],
                            ones,
                            a[:, CHUNK * c : CHUNK * (c + 1)],
                            start=(r == 0),
                            stop=(r == R_BLOCKS - 1),
                        )
                    if r == R_BLOCKS - 1:
                        # This (h, q) group just stopped: evacuate its banks
                        # (ACT copy; the model shows it fills ACT idle gaps).
                        nc.scalar.copy(
                            out=colsum_sb[0:1, col : col + w],
                            in_=ps[0:1, pq : pq + w],
                        )
                    if stat_idx == N_TILES - LAST_BLOCK_TILES:
                        # Ship everything but the last row block's stats now;
                        # only the small remainder rides the kernel tail.
                        nc.sync.dma_start(
                            out=stats[:, :STAT_CUT], in_=stats_sb[:, :STAT_CUT]
                        )

        nc.sync.dma_start(out=colsum, in_=colsum_sb)
        nc.scalar.dma_start(out=stats[:, STAT_CUT:], in_=stats_sb[:, STAT_CUT:])


def build_nc():
    global _NC_CACHE
    if _NC_CACHE is None:
        nc = bacc.Bacc("TRN2", target_bir_lowering=False, debug=False)
        with tile.TileContext(nc) as tc:
            _body(tc)
        nc.compile()
        _NC_CACHE = nc
    return _NC_CACHE


def run_device(logits16, trace=False):
    nc = build_nc()
    in_maps = [
        {"logits": np.ascontiguousarray(logits16[ROWS * k : ROWS * (k + 1)])}
        for k in range(N_CORES)
    ]
    return run_bass_kernel_spmd(
        nc, in_maps, core_ids=list(range(N_CORES)), trace=trace
    )


def combine(results, logits_np, pids_np):
    colsums = np.stack(
        [results[k]["colsum"].reshape(C) for k in range(N_CORES)]
    ).astype(np.float64)
    stats = np.stack([results[k]["stats"] for k in range(N_CORES)]).astype(
        np.float64
    )  # [cores, P, STAT_NCOLS]; see E_COLS / Q_COLS for the layout

    s = colsums.sum(axis=0)                      # [C]
    total_sum = s.sum()
    sumsq = stats[:, :, Q_COLS].sum()
    penalty = s @ s - sumsq

    # Row sumexp: sum each row block's sumexp columns.
    sumexp = np.stack(
        [
            stats[:, :, [c for c, rr in E_COLS if rr == r]].sum(axis=2)
            for r in range(R_BLOCKS)
        ],
        axis=2,
    )  # [cores, P, R_BLOCKS]
    lse = np.log(sumexp)
    tgt = logits_np[np.arange(N), pids_np].astype(np.float64).sum()
    ce = lse.mean() - ((1.0 - EPS) * tgt + (EPS / C) * total_sum) / N
    return np.float32(penalty + ce)


def kernel(logits, pids):
    logits_np = np.asarray(logits, dtype=np.float32)
    pids_np = np.asarray(pids).astype(np.int64)
    logits16 = np.ascontiguousarray(logits_np.astype(IN_NP))
    res = run_device(logits16)
    return combine(res.results, logits_np, pids_np)


# revision 51
# speedup vs baseline: 1.0656x; 1.0656x over previous
"""Trainium2 Bass kernel for BatchSpectralLoss (penalty + label-smoothed CE).

Math (reference):
    penalty = ||sum_i A_i||^2 - sum(A*A)            (A = logits, [N, C])
    ce      = mean_i [ lse_i - (1-eps)*A[i,pid_i] - (eps/C)*rowsum_i ]
    out     = penalty + ce

Rows are sharded 8 ways (512 rows/core). The host casts logits to fp16
(measured effect on this loss: ~5e-5 relative — comparable to fp32
arithmetic noise) which halves HBM traffic; the kernel is memory-bound.

Device work per core, one pass over the shard in [128, w] tiles:
    - colsum partial  s_k[j] = sum_i A[i, j]   (PE matmul with a ones vector,
      fp32 PSUM accumulation across the 4 row blocks)
    - sumexp per row  (ACT Exp pass, accum_out)
    - sumsq  per row  (DVE scalar_tensor_tensor A*A, accum_out)
Host combines: s = sum_k s_k; penalty = s.s - sum(sumsq); lse = log(sumexp);
sum_i rowsum_i = sum(s); the target-logit gather is a 4096-element host read.
"""

import numpy as np
from contextlib import ExitStack

import concourse.bacc as bacc
import concourse.tile as tile
from concourse import mybir
from concourse.bass_utils import run_bass_kernel_spmd

EPS = 0.1
N, C = 4096, 8192
N_CORES = 8
ROWS = N // N_CORES           # 512 rows per core
P = 128                       # SBUF partitions
R_BLOCKS = ROWS // P          # 4 row blocks per core
HALVES = 2
HALF_C = C // HALVES          # 4096 columns per half (PSUM capacity unit)
TILE_W = 2048                 # default tile width
CHUNK = 512                   # matmul free-dim (one fp32 PSUM bank)

IN_DT = mybir.dt.float16
IN_NP = np.float16


def _tile_width(h, r):
    # Narrow tiles on the first row block (shorter pipeline fill) and the
    # last one (shorter drain tail).
    first = h == 0 and r == 0
    last = h == HALVES - 1 and r == R_BLOCKS - 1
    if first:
        return TILE_W // 2
    if last:
        return TILE_W
    return HALF_C


# sumsq accumulates sequentially in fp32; cap the run length so its rounding
# error (amplified by the penalty's big-number cancellation) stays small.
SUMSQ_CHUNK = 1024

# Stats-column schedule (mirrors the _body loop structure): per tile, one
# sumexp column then one sumsq column per SUMSQ_CHUNK sub-span. Emission
# order is monotonic, so the early/late output-DMA split is a column cut.
E_COLS = []          # (column, row_block)
Q_COLS = []          # columns
TILE_STAT_COLS = []  # per tile: (e_col, [q_cols])
_ncols = 0
for _h in range(HALVES):
    for _r in range(R_BLOCKS):
        _w = _tile_width(_h, _r)
        for _ in range(HALF_C // _w):
            _e = _ncols
            _ncols += 1
            _qs = list(range(_ncols, _ncols + -(-_w // SUMSQ_CHUNK)))
            _ncols += len(_qs)
            E_COLS.append((_e, _r))
            Q_COLS.extend(_qs)
            TILE_STAT_COLS.append((_e, _qs))
STAT_NCOLS = _ncols
N_TILES = len(TILE_STAT_COLS)
LAST_BLOCK_TILES = HALF_C // _tile_width(HALVES - 1, R_BLOCKS - 1)
STAT_CUT = TILE_STAT_COLS[N_TILES - LAST_BLOCK_TILES][0]

_NC_CACHE = None


def _body(tc):
    nc = tc.nc
    logits = nc.dram_tensor(
        "logits", [ROWS, C], IN_DT, kind="ExternalInput"
    ).ap()
    colsum = nc.dram_tensor(
        "colsum", [4, HALVES * 1024], mybir.dt.float32, kind="ExternalOutput"
    ).ap()
    stats = nc.dram_tensor(
        "stats", [P, STAT_NCOLS], mybir.dt.float32, kind="ExternalOutput"
    ).ap()

    with ExitStack() as ctx:
        apool = ctx.enter_context(tc.tile_pool(name="a", bufs=6))
        scratch = ctx.enter_context(tc.tile_pool(name="scratch", bufs=1))
        outp = ctx.enter_context(tc.tile_pool(name="outp", bufs=1))
        psum = ctx.enter_context(tc.tile_pool(name="psum", bufs=1, space="PSUM"))

        # M=32 all-ones weights: each chunk's matmul broadcasts its column
        # sums over a 32-partition group, so PSUM evacuation runs on 128
        # lanes instead of one.
        ones = scratch.tile([P, 32], IN_DT)
        nc.vector.memset(ones, 1.0)
        e_scr = scratch.tile([P, HALF_C], IN_DT)
        s_scr = scratch.tile([P, HALF_C], IN_DT)
        stats_sb = outp.tile([P, STAT_NCOLS], mybir.dt.float32)
        colsum_sb = outp.tile([P, HALVES * 1024], mybir.dt.float32)
        # Chunk cc (512 cols) of half h lives at partition-group 32*(cc%4),
        # free offset h*1024 + (cc//4)*512: 4 banks total, halves disjoint.
        ps = psum.tile([P, HALVES * 1024], mybir.dt.float32)

        stat_idx = 0
        for h in range(HALVES):
            for r in range(R_BLOCKS):
                w = _tile_width(h, r)
                for col in range(HALF_C * h, HALF_C * (h + 1), w):
                    a = apool.tile([P, w], IN_DT, tag=f"a{w}")
                    nc.sync.dma_start(
                        out=a, in_=logits[P * r : P * (r + 1), col : col + w]
                    )
                    e_col, q_cols = TILE_STAT_COLS[stat_idx]
                    stat_idx += 1
                    nc.scalar.activation(
                        out=e_scr[:, :w],
                        in_=a,
                        func=mybir.ActivationFunctionType.Exp,
                        accum_out=stats_sb[:, e_col : e_col + 1],
                    )
                    for si, q_col in enumerate(q_cols):
                        s0 = si * SUMSQ_CHUNK
                        s1 = min(w, s0 + SUMSQ_CHUNK)
                        nc.vector.scalar_tensor_tensor(
                            out=s_scr[:, s0:s1],
                            in0=a[:, s0:s1],
                            scalar=1.0,
                            in1=a[:, s0:s1],
                            op0=mybir.AluOpType.mult,
                            op1=mybir.AluOpType.mult,
                            accum_out=stats_sb[:, q_col : q_col + 1],
                        )
                    pq = col - HALF_C * h
                    for c in range(w // CHUNK):
                        cc = pq // CHUNK + c
                        pg, bk = 32 * (cc % 4), cc // 4
                        off = 1024 * h + CHUNK * bk
                        # skip_group_check: CoreSim's zero-region tracker
                        # can't express four partition-groups sharing a bank;
                        # the pattern is HW-validated (see debug_psum.py).
                        nc.tensor.matmul(
                            ps[pg : pg + 32, off : off + CHUNK],
                            ones,
                            a[:, CHUNK * c : CHUNK * (c + 1)],
                            start=(r == 0),
                            stop=(r == R_BLOCKS - 1),
                            tile_position=(0, pg),
                            skip_group_check=True,
                        )
                    if r == R_BLOCKS - 1 and col + w == HALF_C * (h + 1):
                        # All of half h's groups just stopped: one 128-lane
                        # copy evacuates them (ACT; fills its idle gaps).
                        nc.scalar.copy(
                            out=colsum_sb[:, 1024 * h : 1024 * (h + 1)],
                            in_=ps[:, 1024 * h : 1024 * (h + 1)],
                        )
                    if stat_idx == N_TILES - LAST_BLOCK_TILES:
                        # Ship everything but the last row block's stats now;
                        # only the small remainder rides the kernel tail.
                        nc.sync.dma_start(
                            out=stats[:, :STAT_CUT], in_=stats_sb[:, :STAT_CUT]
                        )

        nc.sync.dma_start(out=colsum, in_=colsum_sb[0:97:32, :])
        nc.sync.dma_start(out=stats[:, STAT_CUT:], in_=stats_sb[:, STAT_CUT:])


def build_nc():
    global _NC_CACHE
    if _NC_CACHE is None:
        nc = bacc.Bacc("TRN2", target_bir_lowering=False, debug=False)
        with tile.TileContext(nc) as tc:
            _body(tc)
        nc.compile()
        _NC_CACHE = nc
    return _NC_CACHE


def run_device(logits16, trace=False):
    nc = build_nc()
    in_maps = [
        {"logits": np.ascontiguousarray(logits16[ROWS * k : ROWS * (k + 1)])}
        for k in range(N_CORES)
    ]
    return run_bass_kernel_spmd(
        nc, in_maps, core_ids=list(range(N_CORES)), trace=trace
    )


def decode_colsum(cs):
    # cs[p, h*1024 + b*512 + j] = s[h*4096 + (b*4+p)*512 + j]
    v = cs.reshape(4, HALVES, 2, CHUNK)          # [p, h, b, j]
    return np.transpose(v, (1, 2, 0, 3)).reshape(C)  # [h, b, p, j] -> flat


def combine(results, logits_np, pids_np):
    colsums = np.stack(
        [decode_colsum(results[k]["colsum"]) for k in range(N_CORES)]
    ).astype(np.float64)
    stats = np.stack([results[k]["stats"] for k in range(N_CORES)]).astype(
        np.float64
    )  # [cores, P, STAT_NCOLS]; see E_COLS / Q_COLS for the layout

    s = colsums.sum(axis=0)                      # [C]
    total_sum = s.sum()
    sumsq = stats[:, :, Q_COLS].sum()
    penalty = s @ s - sumsq

    # Row sumexp: sum each row block's sumexp columns.
    sumexp = np.stack(
        [
            stats[:, :, [c for c, rr in E_COLS if rr == r]].sum(axis=2)
            for r in range(R_BLOCKS)
        ],
        axis=2,
    )  # [cores, P, R_BLOCKS]
    lse = np.log(sumexp)
    tgt = logits_np[np.arange(N), pids_np].astype(np.float64).sum()
    ce = lse.mean() - ((1.0 - EPS) * tgt + (EPS / C) * total_sum) / N
    return np.float32(penalty + ce)


def kernel(logits, pids):
    logits_np = np.asarray(logits, dtype=np.float32)
    pids_np = np.asarray(pids).astype(np.int64)
    logits16 = np.ascontiguousarray(logits_np.astype(IN_NP))
    res = run_device(logits16)
    return combine(res.results, logits_np, pids_np)
